# revision 46
# baseline (speedup 1.0000x reference)
# Multi-head masked attention (V = Q source quirk; Wv unused) on 8 TRN2 NeuronCores.
#
# Sharding: 8 cores = 4 batches x 2 head-groups (tensor parallel over heads).
# Core c handles batch b = c//2 and heads g*8..g*8+7 (g = c%2), for ALL 2048
# query positions. Each core projects K^T and Q^T for its heads (contraction
# over the full embedding), derives Q-natural (= V, due to the V=Q source bug)
# from Q^T via DMA XBAR transpose, runs causal attention for all queries, and
# computes a partial output projection against its 512-row slice of Wo^T.
# A pairwise AllReduce (bf16) per 512-query chunk sums the two partials
# on-device; the host just picks rows from the even core of each pair.
#
# Layouts (per core, bf16 matmul operands, fp32 PSUM accumulation):
#   kT  [128=d-in-pair, 4 hp, S]   scores lhsT (head even: partitions 0-63)
#   qT  [128=d-in-pair, 4 hp, S]   scores rhs
#   qn  [128=k-in-tile, S/128, 8*(D+1)]  attnV lhsT; col D of each head slot is
#                                  a ones column -> PSUM partition 64 gathers
#                                  the softmax denominator for free.
#   scores computed transposed (scoresT[k, q] = K @ Q^T) so the softmax sum
#   over keys is a partition-dim reduction done by the ones column on the PE.
#   Causal masking: column-trimmed matmul/exp ranges + one triangular mask
#   multiply on the diagonal 128x128 subtile of each (chunk, k-tile) unit.
#   Softmax normalization: DVE reciprocal of the denominator row + GpSimd
#   partition-broadcast + DVE multiply at PSUM-evict time.

import sys

for _p in ("/opt/trn_rl_repo",):
    if _p not in sys.path:
        sys.path.append(_p)

import numpy as np
import ml_dtypes

BF16 = ml_dtypes.bfloat16

B, S, E, H = 4, 2048, 1024, 16
D = E // H
NCORES = 8

_CACHE = {}


def _build_program(S, E, H, n_cores=NCORES):
    import concourse.bass as bass
    import concourse.mybir as mybir
    import concourse.tile as tile
    from concourse import bacc
    from contextlib import ExitStack

    P = 128
    D = E // H
    assert D == 64 and S % 512 == 0 and E % P == 0
    S_t = S // P            # seq tiles (16)
    E_t = E // P            # embedding chunks (8)
    G = E // 2              # projection width per core (512 = 8 heads)
    G_t = G // P            # local head pairs (4)
    CH = 512                # attention q-chunk
    n_ch = S // CH          # q-chunks (4)
    spc = CH // P           # seq tiles per chunk (4)
    f32 = mybir.dt.float32
    bf16 = mybir.dt.bfloat16
    Exp = mybir.ActivationFunctionType.Exp
    scale = 1.0 / float(np.sqrt(E))
    rgroups = [[2 * i, 2 * i + 1] for i in range(n_cores // 2)]

    nc = bacc.Bacc(
        "TRN2", target_bir_lowering=False, debug=False, num_devices=n_cores
    )

    xT_d = nc.dram_tensor("xT", [E, S], bf16, kind="ExternalInput").ap()
    wqT_d = nc.dram_tensor("wqT", [E, G], bf16, kind="ExternalInput").ap()
    wkT_d = nc.dram_tensor("wkT", [E, G], bf16, kind="ExternalInput").ap()
    woT_d = nc.dram_tensor("woT", [G, E], bf16, kind="ExternalInput").ap()
    bo_d = nc.dram_tensor("bo", [1, E], bf16, kind="ExternalInput").ap()
    tri_d = nc.dram_tensor("tri", [P, P], bf16, kind="ExternalInput").ap()
    out_d = nc.dram_tensor("out", [S, E], bf16, kind="ExternalOutput").ap()
    # collective outputs must be non-IO dram tensors (one per 256-row slab,
    # plus 128-row slabs for the finer-pipelined last chunk)
    bno_d = [
        nc.dram_tensor(f"bno{i}", [2 * P, E], bf16).ap()
        for i in range(S // (2 * P))
    ]
    bnoj_d = [
        nc.dram_tensor(f"bnoj{i}", [P, E], bf16).ap() for i in range(4)
    ]

    with tile.TileContext(nc) as tc, ExitStack() as ctx:
        main = ctx.enter_context(tc.tile_pool(name="main", bufs=1))
        expp = ctx.enter_context(tc.tile_pool(name="expp", bufs=4))
        rsbp = ctx.enter_context(tc.tile_pool(name="rsbp", bufs=2))
        bcp = ctx.enter_context(tc.tile_pool(name="bcp", bufs=2))
        attnp = ctx.enter_context(tc.tile_pool(name="attnp", bufs=2))
        poutp = ctx.enter_context(tc.tile_pool(name="poutp", bufs=2))
        drp = ctx.enter_context(tc.tile_pool(name="drp", bufs=3, space="DRAM"))

        kT = main.tile([P, G_t, S], bf16)
        # qT split per s-chunk so the XBAR transposes (tile-granular deps)
        # can start as soon as their chunk is projected
        qTs = []
        for sc in range(S // CH):
            qTc = main.tile([P, G_t, CH], bf16, tag=f"qT{sc}")
            qTs.append(qTc)
        qn = main.tile([P, S_t, 2 * G_t * (D + 1)], bf16)
        wo_sb = main.tile([P, G_t, E], bf16)
        bias_sb = main.tile([P, E], bf16)
        bo_sb = main.tile([1, E], bf16)
        tri_sb = main.tile([P, P], bf16)

        qn4 = qn.rearrange("p t (h c) -> p t h c", c=D + 1)

        nc.vector.memset(qn4[:, :, :, D:D + 1], 1.0)

        # ---- phase 1: K^T / Q^T projections (s-chunked for early start) ----
        with tc.tile_pool(name="ph1", bufs=1) as ph1, \
                tc.tile_pool(name="pproj", bufs=3, space="PSUM") as pproj:
            xT_r = xT_d.rearrange("(t p) s -> p t s", p=P)
            wq_r = wqT_d.rearrange("(t p) d -> p t d", p=P)
            wk_r = wkT_d.rearrange("(t p) d -> p t d", p=P)
            wqs, wks, xs = [], [], {}
            for e in range(E_t):
                wke = ph1.tile([P, G], bf16, tag=f"wk{e}")
                nc.sync.dma_start(out=wke, in_=wk_r[:, e, :])
                wks.append(wke)
            for e in range(E_t):
                xe = ph1.tile([P, CH], bf16, tag=f"x{e}_0")
                nc.sync.dma_start(out=xe, in_=xT_r[:, e, 0:CH])
                xs[(e, 0)] = xe
            for e in range(E_t):
                wqe = ph1.tile([P, G], bf16, tag=f"wq{e}")
                nc.sync.dma_start(out=wqe, in_=wq_r[:, e, :])
                wqs.append(wqe)
            for sc in range(1, S // CH):
                for e in range(E_t):
                    xe = ph1.tile([P, CH], bf16, tag=f"x{e}_{sc}")
                    nc.sync.dma_start(
                        out=xe, in_=xT_r[:, e, sc * CH:(sc + 1) * CH]
                    )
                    xs[(e, sc)] = xe
            # lower-priority loads, after the projection-critical DMAs
            nc.sync.dma_start(out=tri_sb, in_=tri_d)
            nc.sync.dma_start(out=bo_sb, in_=bo_d)
            wo_r = woT_d.rearrange("(c p) e -> p c e", p=P)
            for cp in range(G_t):
                nc.sync.dma_start(out=wo_sb[:, cp, :], in_=wo_r[:, cp, :])
            nc.gpsimd.partition_broadcast(bias_sb, bo_sb)
            for sc in range(S // CH):
                sl = slice(sc * CH, (sc + 1) * CH)
                tsl = slice(sc * spc, (sc + 1) * spc)
                for hp in range(G_t):
                    ps = pproj.tile([P, CH], f32)
                    for e in range(E_t):
                        nc.tensor.matmul(
                            ps,
                            wks[e][:, hp * P:(hp + 1) * P],
                            xs[(e, sc)],
                            start=(e == 0),
                            stop=(e == E_t - 1),
                        )
                    nc.scalar.copy(out=kT[:, hp, sl], in_=ps)
                for hp in range(G_t):
                    ps = pproj.tile([P, CH], f32)
                    for e in range(E_t):
                        nc.tensor.matmul(
                            ps,
                            wqs[e][:, hp * P:(hp + 1) * P],
                            xs[(e, sc)],
                            start=(e == 0),
                            stop=(e == E_t - 1),
                        )
                    nc.vector.tensor_copy(out=qTs[sc][:, hp, :], in_=ps)
                # qn (= V natural): direct projection, strided evict into
                # the 65-col head slots
                for stl in range(spc):
                    st = sc * spc + stl
                    ps = pproj.tile([P, CH], f32)
                    for e in range(E_t):
                        nc.tensor.matmul(
                            ps,
                            xs[(e, sc)][:, stl * P:(stl + 1) * P],
                            wqs[e],
                            start=(e == 0),
                            stop=(e == E_t - 1),
                        )
                    if st % 2 == 0:
                        nc.vector.tensor_copy(
                            out=qn4[:, st, :, 0:D],
                            in_=ps.rearrange("p (h c) -> p h c", c=D),
                        )
                    else:
                        nc.scalar.copy(
                            out=qn4[:, st, :, 0:D],
                            in_=ps.rearrange("p (h c) -> p h c", c=D),
                        )

        # ---- phase 2: attention + partial out-projection, per q-chunk ----
        psc = ctx.enter_context(tc.tile_pool(name="psc", bufs=2, space="PSUM"))
        pav = ctx.enter_context(tc.tile_pool(name="pav", bufs=1, space="PSUM"))
        outp = ctx.enter_context(tc.tile_pool(name="outp", bufs=1, space="PSUM"))
        def emit_bounce(qc, pout):
            # bounce the partials to DRAM right away (spread across queues)
            bncs = []
            for hh in range(2):
                bnc = drp.tile([2 * P, E], bf16, tag="bnc_in")
                bnc_r = bnc.rearrange("(j p) e -> p j e", p=P)
                for jj in range(2):
                    jt = 2 * hh + jj
                    for eh in range(2):
                        el = slice(eh * CH, (eh + 1) * CH)
                        nc.sync.dma_start(
                            out=bnc_r[:, jj, el], in_=pout[:, jt, el]
                        )
                bncs.append(bnc)
            return bncs

        def emit_output_block(qc, bncs):
            # pairwise AllReduce + final rows, per half-chunk
            for hh in range(2):
                bnc = bncs[hh]
                bno = bno_d[2 * qc + hh]
                nc.gpsimd.collective_compute(
                    "AllReduce",
                    mybir.AluOpType.add,
                    replica_groups=rgroups,
                    ins=[bnc.opt()],
                    outs=[bno.opt()],
                )
                r0 = qc * CH + hh * 2 * P
                bno_r = bno.rearrange("(j p) e -> p j e", p=P)
                out_r = out_d[r0:r0 + 2 * P, :].rearrange(
                    "(j p) e -> p j e", p=P
                )
                for jj in range(2):
                    nc.sync.dma_start(
                        out=out_r[:, jj, :], in_=bno_r[:, jj, :]
                    )

        pending = None
        for qc in range(n_ch):
            attnT = attnp.tile([P, G_t, CH], bf16)
            t_max = spc * qc + spc - 1
            for hp in range(G_t):
                # emit the previous chunk's AllReduce block mid-loop so the
                # GpSimd queue reaches it after its input bounce completed
                # (a waiting collective blocks the broadcasts behind it)
                if hp == 2 and pending is not None:
                    emit_output_block(*pending)
                    pending = None
                hA, hB = 2 * hp, 2 * hp + 1
                k0 = (qc * G_t + hp) * 2
                pvA = pav.tile([P, CH], f32, tag=f"pv{k0 % 3}")
                pvB = pav.tile([P, CH], f32, tag=f"pv{(k0 + 1) % 3}")
                pvs = [pvA, pvB]
                for t in range(t_max + 1):
                    jloc = max(0, t - spc * qc)
                    qoff = jloc * P
                    ex = expp.tile([P, 2, CH], bf16)
                    sct = psc.tile([P, 2, CH], f32)
                    for h2 in range(2):
                        nc.tensor.matmul(
                            sct[:, h2, qoff:CH],
                            kT[h2 * D:(h2 + 1) * D, hp, t * P:(t + 1) * P],
                            qTs[qc][h2 * D:(h2 + 1) * D, hp, qoff:CH],
                            start=True,
                            stop=True,
                        )
                    nc.scalar.activation(
                        out=ex[:, :, qoff:CH],
                        in_=sct[:, :, qoff:CH],
                        func=Exp,
                        scale=scale,
                    )
                    if t >= spc * qc:  # diagonal subtile: causal mask
                        for h2 in range(2):
                            nc.vector.tensor_mul(
                                out=ex[:, h2, qoff:qoff + P],
                                in0=ex[:, h2, qoff:qoff + P],
                                in1=tri_sb,
                            )
                    for h2, h in ((0, hA), (1, hB)):
                        nc.tensor.matmul(
                            pvs[h2][0:D + 1, qoff:CH],
                            qn4[:, t, h, :],
                            ex[:, h2, qoff:CH],
                            start=(t == 0),
                            stop=(t == t_max),
                        )
                # evict unnormalized fast (frees PSUM for the next head
                # pair), then normalize attnT in place off the PE path:
                # fast DVE reciprocal, GpSimd broadcast, DVE multiply
                for half in range(2):
                    pv = pvs[half]
                    dst = attnT[half * D:(half + 1) * D, hp, :]
                    stg = rsbp.tile([1, CH], f32, tag="stg")
                    nc.vector.tensor_copy(out=stg, in_=pv[D:D + 1, :])
                    nc.vector.tensor_copy(out=dst, in_=pv[0:D, :])
                    rsb = rsbp.tile([1, CH], f32, tag="rsb")
                    nc.vector.reciprocal_approx_fast(out=rsb, in_=stg)
                    bc = bcp.tile([P, CH], f32)
                    nc.gpsimd.partition_broadcast(bc, rsb)
                    nc.vector.tensor_mul(
                        out=dst, in0=dst,
                        in1=bc[half * D:(half + 1) * D, :],
                    )
            # partial out-projection for this q-chunk (contraction over my
            # 512 attn dims; pair partner contributes the other 512)
            pout = poutp.tile([P, spc, E], bf16)
            for jt in range(spc):
                for ec in range(E // CH):
                    ps = outp.tile([P, CH], f32)
                    for cp in range(G_t):
                        nc.tensor.matmul(
                            ps,
                            attnT[:, cp, jt * P:(jt + 1) * P],
                            wo_sb[:, cp, ec * CH:(ec + 1) * CH],
                            start=(cp == 0),
                            stop=(cp == G_t - 1),
                        )
                    # fold the (half) bias into the PSUM evict
                    nc.vector.tensor_add(
                        out=pout[:, jt, ec * CH:(ec + 1) * CH],
                        in0=ps,
                        in1=bias_sb[:, ec * CH:(ec + 1) * CH],
                    )
            pending = (qc, emit_bounce(qc, pout))
        emit_output_block(*pending)

    nc.finalize()
    return nc


def _prep_inputs(x, Wk, Wq, Wo, bo, n_cores=NCORES):
    """Per-core input maps: batch c//2, head-group c%2 (bf16 operands)."""
    b, s, e = x.shape
    g = e // 2
    wqT = np.ascontiguousarray(Wq.T).astype(BF16)
    wkT = np.ascontiguousarray(Wk.T).astype(BF16)
    woT = np.ascontiguousarray(Wo.T).astype(BF16)
    bo_half = (bo.reshape(1, e) * 0.5).astype(BF16)
    tri = np.triu(np.ones((128, 128), dtype=np.float32)).astype(BF16)
    xTs = [np.ascontiguousarray(x[bi].T).astype(BF16) for bi in range(b)]
    in_maps = []
    for c in range(n_cores):
        bi, hg = c // 2, c % 2
        in_maps.append(
            {
                "xT": xTs[bi],
                "wqT": np.ascontiguousarray(wqT[:, hg * g:(hg + 1) * g]),
                "wkT": np.ascontiguousarray(wkT[:, hg * g:(hg + 1) * g]),
                "woT": np.ascontiguousarray(woT[hg * g:(hg + 1) * g, :]),
                "bo": bo_half,
                "tri": tri,
            }
        )
    return in_maps


def _gather(res, b, s, e):
    """Full output from per-core results (even core of each pair has all rows)."""
    out = np.empty((b, s, e), dtype=np.float32)
    for bi in range(b):
        out[bi] = res.results[2 * bi]["out"].astype(np.float32)
    return out


def kernel(x, Wk, Wq, Wv, Wo, bo):
    from concourse import bass_utils

    x = np.asarray(x, dtype=np.float32)
    Wk = np.asarray(Wk, dtype=np.float32)
    Wq = np.asarray(Wq, dtype=np.float32)
    Wo = np.asarray(Wo, dtype=np.float32)
    bo = np.asarray(bo, dtype=np.float32)
    b, s, e = x.shape
    key = (s, e, H)
    if key not in _CACHE:
        _CACHE[key] = _build_program(s, e, H)
    nc = _CACHE[key]
    in_maps = _prep_inputs(x, Wk, Wq, Wo, bo)
    res = bass_utils.run_bass_kernel_spmd(nc, in_maps, list(range(NCORES)))
    return _gather(res, b, s, e)


if __name__ == "__main__":
    nc = _build_program(S, E, H)
    print("built ok")


# revision 47
# speedup vs baseline: 1.1917x; 1.1917x over previous
# Multi-head masked attention (V = Q source quirk; Wv unused) on 8 TRN2 NeuronCores.
#
# Sharding: 8 cores = 4 batches x 2 head-groups (tensor parallel over heads).
# Core c handles batch b = c//2 and heads g*8..g*8+7 (g = c%2), for ALL 2048
# query positions. Each core projects K^T and Q^T for its heads (contraction
# over the full embedding), derives Q-natural (= V, due to the V=Q source bug)
# from Q^T via DMA XBAR transpose, runs causal attention for all queries, and
# computes a partial output projection against its 512-row slice of Wo^T.
# A pairwise AllReduce (bf16) per 512-query chunk sums the two partials
# on-device; the host just picks rows from the even core of each pair.
#
# Layouts (per core, bf16 matmul operands, fp32 PSUM accumulation):
#   kT  [128=d-in-pair, 4 hp, S]   scores lhsT (head even: partitions 0-63)
#   qT  [128=d-in-pair, 4 hp, S]   scores rhs
#   qn  [128=k-in-tile, S/128, 8*(D+1)]  attnV lhsT; col D of each head slot is
#                                  a ones column -> PSUM partition 64 gathers
#                                  the softmax denominator for free.
#   scores computed transposed (scoresT[k, q] = K @ Q^T) so the softmax sum
#   over keys is a partition-dim reduction done by the ones column on the PE.
#   Causal masking: column-trimmed matmul/exp ranges + one triangular mask
#   multiply on the diagonal 128x128 subtile of each (chunk, k-tile) unit.
#   Softmax normalization: DVE reciprocal of the denominator row + GpSimd
#   partition-broadcast + DVE multiply at PSUM-evict time.

import sys

for _p in ("/opt/trn_rl_repo",):
    if _p not in sys.path:
        sys.path.append(_p)

import numpy as np
import ml_dtypes

BF16 = ml_dtypes.bfloat16

B, S, E, H = 4, 2048, 1024, 16
D = E // H
NCORES = 8

_CACHE = {}


def _build_program(S, E, H, n_cores=NCORES):
    import concourse.bass as bass
    import concourse.mybir as mybir
    import concourse.tile as tile
    from concourse import bacc
    from contextlib import ExitStack

    P = 128
    D = E // H
    assert D == 64 and S % 512 == 0 and E % P == 0
    S_t = S // P            # seq tiles (16)
    E_t = E // P            # embedding chunks (8)
    G = E // 2              # projection width per core (512 = 8 heads)
    G_t = G // P            # local head pairs (4)
    CH = 512                # attention q-chunk
    n_ch = S // CH          # q-chunks (4)
    spc = CH // P           # seq tiles per chunk (4)
    f32 = mybir.dt.float32
    bf16 = mybir.dt.bfloat16
    Exp = mybir.ActivationFunctionType.Exp
    scale = 1.0 / float(np.sqrt(E))
    rgroups = [[2 * i, 2 * i + 1] for i in range(n_cores // 2)]

    nc = bacc.Bacc(
        "TRN2", target_bir_lowering=False, debug=False, num_devices=n_cores
    )

    xT_d = nc.dram_tensor("xT", [E, S], bf16, kind="ExternalInput").ap()
    wqT_d = nc.dram_tensor("wqT", [E, G], bf16, kind="ExternalInput").ap()
    wkT_d = nc.dram_tensor("wkT", [E, G], bf16, kind="ExternalInput").ap()
    woT_d = nc.dram_tensor("woT", [G, E], bf16, kind="ExternalInput").ap()
    bo_d = nc.dram_tensor("bo", [1, E], bf16, kind="ExternalInput").ap()
    tri_d = nc.dram_tensor("tri", [P, P], bf16, kind="ExternalInput").ap()
    out_d = nc.dram_tensor("out", [S, E], bf16, kind="ExternalOutput").ap()
    # collective outputs must be non-IO dram tensors (one per 256-row slab,
    # plus 128-row slabs for the finer-pipelined last chunk)
    bno_d = [
        nc.dram_tensor(f"bno{i}", [2 * P, E], bf16).ap()
        for i in range(S // (2 * P))
    ]
    bnoj_d = [
        nc.dram_tensor(f"bnoj{i}", [P, E], bf16).ap() for i in range(4)
    ]

    with tile.TileContext(nc) as tc, ExitStack() as ctx:
        main = ctx.enter_context(tc.tile_pool(name="main", bufs=1))
        expp = ctx.enter_context(tc.tile_pool(name="expp", bufs=4))
        rsbp = ctx.enter_context(tc.tile_pool(name="rsbp", bufs=2))
        bcp = ctx.enter_context(tc.tile_pool(name="bcp", bufs=2))
        attnp = ctx.enter_context(tc.tile_pool(name="attnp", bufs=2))
        poutp = ctx.enter_context(tc.tile_pool(name="poutp", bufs=2))
        drp = ctx.enter_context(tc.tile_pool(name="drp", bufs=3, space="DRAM"))

        kT = main.tile([P, G_t, S], bf16)
        # qT split per s-chunk so the XBAR transposes (tile-granular deps)
        # can start as soon as their chunk is projected
        qTs = []
        for sc in range(S // CH):
            qTc = main.tile([P, G_t, CH], bf16, tag=f"qT{sc}")
            qTs.append(qTc)
        qn = main.tile([P, S_t, 2 * G_t * (D + 1)], bf16)
        wo_sb = main.tile([P, G_t, E], bf16)
        bias_sb = main.tile([P, E], bf16)
        bo_sb = main.tile([1, E], bf16)
        tri_sb = main.tile([P, P], bf16)

        qn4 = qn.rearrange("p t (h c) -> p t h c", c=D + 1)

        nc.vector.memset(qn4[:, :, :, D:D + 1], 1.0)

        # ---- phase 1: K^T / Q^T projections (s-chunked for early start) ----
        with tc.tile_pool(name="ph1", bufs=1) as ph1, \
                tc.tile_pool(name="pproj", bufs=3, space="PSUM") as pproj:
            xT_r = xT_d.rearrange("(t p) s -> p t s", p=P)
            wq_r = wqT_d.rearrange("(t p) d -> p t d", p=P)
            wk_r = wkT_d.rearrange("(t p) d -> p t d", p=P)
            wqs, wks, xs = [], [], {}
            for e in range(E_t):
                wke = ph1.tile([P, G], bf16, tag=f"wk{e}")
                nc.sync.dma_start(out=wke, in_=wk_r[:, e, :])
                wks.append(wke)
            for e in range(E_t):
                xe = ph1.tile([P, CH], bf16, tag=f"x{e}_0")
                nc.sync.dma_start(out=xe, in_=xT_r[:, e, 0:CH])
                xs[(e, 0)] = xe
            for e in range(E_t):
                wqe = ph1.tile([P, G], bf16, tag=f"wq{e}")
                nc.sync.dma_start(out=wqe, in_=wq_r[:, e, :])
                wqs.append(wqe)
            for sc in range(1, S // CH):
                for e in range(E_t):
                    xe = ph1.tile([P, CH], bf16, tag=f"x{e}_{sc}")
                    nc.sync.dma_start(
                        out=xe, in_=xT_r[:, e, sc * CH:(sc + 1) * CH]
                    )
                    xs[(e, sc)] = xe
            # lower-priority loads, after the projection-critical DMAs
            nc.sync.dma_start(out=tri_sb, in_=tri_d)
            nc.sync.dma_start(out=bo_sb, in_=bo_d)
            wo_r = woT_d.rearrange("(c p) e -> p c e", p=P)
            for cp in range(G_t):
                nc.sync.dma_start(out=wo_sb[:, cp, :], in_=wo_r[:, cp, :])
            nc.gpsimd.partition_broadcast(bias_sb, bo_sb)
            for sc in range(S // CH):
                sl = slice(sc * CH, (sc + 1) * CH)
                tsl = slice(sc * spc, (sc + 1) * spc)
                for hp in range(G_t):
                    ps = pproj.tile([P, CH], f32)
                    for e in range(E_t):
                        nc.tensor.matmul(
                            ps,
                            wks[e][:, hp * P:(hp + 1) * P],
                            xs[(e, sc)],
                            start=(e == 0),
                            stop=(e == E_t - 1),
                        )
                    nc.scalar.copy(out=kT[:, hp, sl], in_=ps)
                for hp in range(G_t):
                    ps = pproj.tile([P, CH], f32)
                    for e in range(E_t):
                        nc.tensor.matmul(
                            ps,
                            wqs[e][:, hp * P:(hp + 1) * P],
                            xs[(e, sc)],
                            start=(e == 0),
                            stop=(e == E_t - 1),
                        )
                    nc.vector.tensor_copy(out=qTs[sc][:, hp, :], in_=ps)
                # qn (= V natural): direct projection, strided evict into
                # the 65-col head slots
                for stl in range(spc):
                    st = sc * spc + stl
                    ps = pproj.tile([P, CH], f32)
                    for e in range(E_t):
                        nc.tensor.matmul(
                            ps,
                            xs[(e, sc)][:, stl * P:(stl + 1) * P],
                            wqs[e],
                            start=(e == 0),
                            stop=(e == E_t - 1),
                        )
                    nc.vector.tensor_copy(
                        out=qn4[:, st, :, 0:D],
                        in_=ps.rearrange("p (h c) -> p h c", c=D),
                    )

        # ---- phase 2: attention + partial out-projection, per q-chunk ----
        psc = ctx.enter_context(tc.tile_pool(name="psc", bufs=2, space="PSUM"))
        pav = ctx.enter_context(tc.tile_pool(name="pav", bufs=1, space="PSUM"))
        outp = ctx.enter_context(tc.tile_pool(name="outp", bufs=1, space="PSUM"))
        def emit_bounce(qc, pout):
            # bounce the partials to DRAM right away (spread across queues)
            bncs = []
            for hh in range(2):
                bnc = drp.tile([2 * P, E], bf16, tag="bnc_in")
                bnc_r = bnc.rearrange("(j p) e -> p j e", p=P)
                for jj in range(2):
                    jt = 2 * hh + jj
                    for eh in range(2):
                        el = slice(eh * CH, (eh + 1) * CH)
                        nc.sync.dma_start(
                            out=bnc_r[:, jj, el], in_=pout[:, jt, el]
                        )
                bncs.append(bnc)
            return bncs

        def emit_output_block(qc, bncs):
            # pairwise AllReduce + final rows, per half-chunk
            for hh in range(2):
                bnc = bncs[hh]
                bno = bno_d[2 * qc + hh]
                nc.gpsimd.collective_compute(
                    "AllReduce",
                    mybir.AluOpType.add,
                    replica_groups=rgroups,
                    ins=[bnc.opt()],
                    outs=[bno.opt()],
                )
                r0 = qc * CH + hh * 2 * P
                bno_r = bno.rearrange("(j p) e -> p j e", p=P)
                out_r = out_d[r0:r0 + 2 * P, :].rearrange(
                    "(j p) e -> p j e", p=P
                )
                for jj in range(2):
                    nc.sync.dma_start(
                        out=out_r[:, jj, :], in_=bno_r[:, jj, :]
                    )

        pending = None
        for qc in range(n_ch):
            attnT = attnp.tile([P, G_t, CH], bf16)
            t_max = spc * qc + spc - 1
            for hp in range(G_t):
                # emit the previous chunk's AllReduce block mid-loop so the
                # GpSimd queue reaches it after its input bounce completed
                # (a waiting collective blocks the broadcasts behind it)
                if hp == 2 and pending is not None:
                    emit_output_block(*pending)
                    pending = None
                hA, hB = 2 * hp, 2 * hp + 1
                k0 = (qc * G_t + hp) * 2
                pvA = pav.tile([P, CH], f32, tag=f"pv{k0 % 3}")
                pvB = pav.tile([P, CH], f32, tag=f"pv{(k0 + 1) % 3}")
                pvs = [pvA, pvB]
                for t in range(t_max + 1):
                    jloc = max(0, t - spc * qc)
                    qoff = jloc * P
                    ex = expp.tile([P, 2, CH], bf16)
                    sct = psc.tile([P, 2, CH], f32)
                    for h2 in range(2):
                        nc.tensor.matmul(
                            sct[:, h2, qoff:CH],
                            kT[h2 * D:(h2 + 1) * D, hp, t * P:(t + 1) * P],
                            qTs[qc][h2 * D:(h2 + 1) * D, hp, qoff:CH],
                            start=True,
                            stop=True,
                        )
                    nc.scalar.activation(
                        out=ex[:, :, qoff:CH],
                        in_=sct[:, :, qoff:CH],
                        func=Exp,
                        scale=scale,
                    )
                    if t >= spc * qc:  # diagonal subtile: causal mask
                        for h2 in range(2):
                            nc.vector.tensor_mul(
                                out=ex[:, h2, qoff:qoff + P],
                                in0=ex[:, h2, qoff:qoff + P],
                                in1=tri_sb,
                            )
                    for h2, h in ((0, hA), (1, hB)):
                        nc.tensor.matmul(
                            pvs[h2][0:D + 1, qoff:CH],
                            qn4[:, t, h, :],
                            ex[:, h2, qoff:CH],
                            start=(t == 0),
                            stop=(t == t_max),
                        )
                # evict unnormalized fast (frees PSUM for the next head
                # pair), then normalize attnT in place off the PE path:
                # fast DVE reciprocal, GpSimd broadcast, DVE multiply
                for half in range(2):
                    pv = pvs[half]
                    dst = attnT[half * D:(half + 1) * D, hp, :]
                    stg = rsbp.tile([1, CH], f32, tag="stg")
                    nc.vector.tensor_copy(out=stg, in_=pv[D:D + 1, :])
                    nc.vector.tensor_copy(out=dst, in_=pv[0:D, :])
                    rsb = rsbp.tile([1, CH], f32, tag="rsb")
                    nc.vector.reciprocal_approx_fast(out=rsb, in_=stg)
                    bc = bcp.tile([P, CH], f32)
                    nc.gpsimd.partition_broadcast(bc, rsb)
                    nc.vector.tensor_mul(
                        out=dst, in0=dst,
                        in1=bc[half * D:(half + 1) * D, :],
                    )
            # partial out-projection for this q-chunk (contraction over my
            # 512 attn dims; pair partner contributes the other 512)
            pout = poutp.tile([P, spc, E], bf16)
            for jt in range(spc):
                for ec in range(E // CH):
                    ps = outp.tile([P, CH], f32)
                    for cp in range(G_t):
                        nc.tensor.matmul(
                            ps,
                            attnT[:, cp, jt * P:(jt + 1) * P],
                            wo_sb[:, cp, ec * CH:(ec + 1) * CH],
                            start=(cp == 0),
                            stop=(cp == G_t - 1),
                        )
                    # fold the (half) bias into the PSUM evict
                    nc.vector.tensor_add(
                        out=pout[:, jt, ec * CH:(ec + 1) * CH],
                        in0=ps,
                        in1=bias_sb[:, ec * CH:(ec + 1) * CH],
                    )
            pending = (qc, emit_bounce(qc, pout))
        emit_output_block(*pending)

    nc.finalize()
    return nc


def _prep_inputs(x, Wk, Wq, Wo, bo, n_cores=NCORES):
    """Per-core input maps: batch c//2, head-group c%2 (bf16 operands)."""
    b, s, e = x.shape
    g = e // 2
    wqT = np.ascontiguousarray(Wq.T).astype(BF16)
    wkT = np.ascontiguousarray(Wk.T).astype(BF16)
    woT = np.ascontiguousarray(Wo.T).astype(BF16)
    bo_half = (bo.reshape(1, e) * 0.5).astype(BF16)
    tri = np.triu(np.ones((128, 128), dtype=np.float32)).astype(BF16)
    xTs = [np.ascontiguousarray(x[bi].T).astype(BF16) for bi in range(b)]
    in_maps = []
    for c in range(n_cores):
        bi, hg = c // 2, c % 2
        in_maps.append(
            {
                "xT": xTs[bi],
                "wqT": np.ascontiguousarray(wqT[:, hg * g:(hg + 1) * g]),
                "wkT": np.ascontiguousarray(wkT[:, hg * g:(hg + 1) * g]),
                "woT": np.ascontiguousarray(woT[hg * g:(hg + 1) * g, :]),
                "bo": bo_half,
                "tri": tri,
            }
        )
    return in_maps


def _gather(res, b, s, e):
    """Full output from per-core results (even core of each pair has all rows)."""
    out = np.empty((b, s, e), dtype=np.float32)
    for bi in range(b):
        out[bi] = res.results[2 * bi]["out"].astype(np.float32)
    return out


def kernel(x, Wk, Wq, Wv, Wo, bo):
    from concourse import bass_utils

    x = np.asarray(x, dtype=np.float32)
    Wk = np.asarray(Wk, dtype=np.float32)
    Wq = np.asarray(Wq, dtype=np.float32)
    Wo = np.asarray(Wo, dtype=np.float32)
    bo = np.asarray(bo, dtype=np.float32)
    b, s, e = x.shape
    key = (s, e, H)
    if key not in _CACHE:
        _CACHE[key] = _build_program(s, e, H)
    nc = _CACHE[key]
    in_maps = _prep_inputs(x, Wk, Wq, Wo, bo)
    res = bass_utils.run_bass_kernel_spmd(nc, in_maps, list(range(NCORES)))
    return _gather(res, b, s, e)


if __name__ == "__main__":
    nc = _build_program(S, E, H)
    print("built ok")
